# revision 1
# baseline (speedup 1.0000x reference)
"""GATv2 (3-layer, PyG semantics) + global mean pool + MLP on 8 trn2 NeuronCores.

Self-contained: hardcodes problem shapes from nn_GAT_47906065219807.
Sharding: data-parallel over contiguous node ranges (2500 nodes/core); each
core owns edges whose dst lands in its range (edges cross shards via an
AllGather of the source-side projections).
"""
import numpy as np
import ml_dtypes

import concourse.bacc as bacc
import concourse.mybir as mybir
import concourse.tile as tile
from concourse.bass_utils import run_bass_kernel_spmd

# problem constants
N_NODES = 20000
N_EDGES = 120000
N_GRAPHS = 512
F_IN = 300
NHID = 256
NOUT = 768
SLOPE = 0.2
EPS = 1e-16

NCORES = 8
NLOC = N_NODES // NCORES            # 2500
NPAD = 2560                         # 20 tiles of 128
NWIN = NPAD // 128                  # 20 windows / node tiles per core
KIN_PAD = 384                       # F_IN padded to 3*128

# per-layer dims: (K_in_padded, F_out, heads, concat)
LAYERS = [
    (KIN_PAD, 4 * NHID, 4, True),
    (4 * NHID, 4 * NHID, 4, True),
    (4 * NHID, 6 * NHID, 6, False),
]
# exp shift per layer (bias added inside exp); scores are O(1) for this data
EXP_SHIFT = [0.0, 0.0, 0.0]

_BF = ml_dtypes.bfloat16
_PROG_CACHE = {}


def _bf16(a):
    return np.ascontiguousarray(a.astype(_BF)).view(np.uint16)


def _wrap_idx(flat_idx):
    """dma_gather index layout: slot i -> [i % 16, i // 16], replicated to
    128 partitions (8 Q7 cores x 16)."""
    n = flat_idx.shape[0]
    assert n % 16 == 0
    w = flat_idx.reshape(n // 16, 16).T.astype(np.int16)
    return np.tile(w, (8, 1)).copy()


def _preprocess(inputs):
    """Host-side sharding/layout. Returns (T, in_maps)."""
    x = np.asarray(inputs["x"], np.float32)
    ei = np.asarray(inputs["edge_index"]).astype(np.int64)
    batch = np.asarray(inputs["batch"]).astype(np.int64)

    loops = np.arange(N_NODES, dtype=np.int64)
    src = np.concatenate([ei[0], loops])
    dst = np.concatenate([ei[1], loops])

    # padded-global row index into the AllGather output [NCORES*NPAD, F]
    src_pad = (src // NLOC) * NPAD + (src % NLOC)

    core_of = dst // NLOC
    # per (core, window) edge buckets
    buckets = [[[] for _ in range(NWIN)] for _ in range(NCORES)]
    order = np.argsort(dst, kind="stable")
    for e in order:
        c = core_of[e]
        dl = dst[e] - c * NLOC
        buckets[c][dl // 128].append(e)
    Tw = tuple(
        max((len(buckets[c][w]) + 127) // 128 for c in range(NCORES))
        for w in range(NWIN)
    )
    Soff = np.concatenate([[0], np.cumsum([t * 128 for t in Tw])])

    # graph counts -> reciprocal (reference divides by max(cnt,1))
    cnt = np.bincount(batch, minlength=N_GRAPHS).astype(np.float32)
    rcnt = 1.0 / np.maximum(cnt, 1.0)

    # shared (per-core identical) tensors
    def wT_pad(w, kpad):
        # host layout [128, KB, F]: [p, b, f] = w[f-major...]  w is [F, K]
        wt = w.T.astype(np.float32)                    # [K, F]
        K, F = wt.shape
        out = np.zeros((kpad, F), np.float32)
        out[:K] = wt
        return _bf16(out.reshape(kpad // 128, 128, F).transpose(1, 0, 2)
                     .reshape(128, (kpad // 128) * F))

    shared = {
        "w1l": wT_pad(np.asarray(inputs["c1_wl"]), KIN_PAD),
        "w1r": wT_pad(np.asarray(inputs["c1_wr"]), KIN_PAD),
        "w2l": wT_pad(np.asarray(inputs["c2_wl"]), 4 * NHID),
        "w2r": wT_pad(np.asarray(inputs["c2_wr"]), 4 * NHID),
        "w3l": wT_pad(np.asarray(inputs["c3_wl"]), 4 * NHID),
        "w3r": wT_pad(np.asarray(inputs["c3_wr"]), 4 * NHID),
        "att1": _bf16(np.tile(np.asarray(inputs["c1_att"]).reshape(1, -1), (128, 1))),
        "att2": _bf16(np.tile(np.asarray(inputs["c2_att"]).reshape(1, -1), (128, 1))),
        "att3": _bf16(np.tile(np.asarray(inputs["c3_att"]).reshape(1, -1), (128, 1))),
        "b1": np.tile(np.asarray(inputs["c1_b"], np.float32).reshape(1, -1), (128, 1)),
        "b2": np.tile(np.asarray(inputs["c2_b"], np.float32).reshape(1, -1), (128, 1)),
        "b3": np.tile(np.asarray(inputs["c3_b"], np.float32).reshape(1, -1), (128, 1)),
        "rcnt": np.tile(rcnt.reshape(1, -1), (128, 1)).astype(np.float32),
        "wfc1": wT_pad(np.asarray(inputs["fp1_w"]), 256),
        "wfc2": wT_pad(np.asarray(inputs["fp2_w"]), 256),
        "bfc1": np.asarray(inputs["fp1_b"], np.float32).reshape(2, 128).T.copy(),
        "bfc2": np.tile(np.asarray(inputs["fp2_b"], np.float32).reshape(1, -1),
                        (128, 1)),
        "shifts": np.tile(np.asarray(EXP_SHIFT + [0.0], np.float32).reshape(1, -1),
                          (128, 1)),
    }

    in_maps = []
    for c in range(NCORES):
        xc = np.zeros((NPAD, KIN_PAD), np.float32)
        xc[:NLOC, :F_IN] = x[c * NLOC:(c + 1) * NLOC]
        # xT host layout [128, 3, NPAD]
        xT = xc.T.reshape(KIN_PAD // 128, 128, NPAD).transpose(1, 0, 2)
        xT = _bf16(xT.reshape(128, (KIN_PAD // 128) * NPAD))

        tot = int(Soff[-1])
        isrc = np.zeros(tot, np.int64)
        idst = np.zeros(tot, np.int64)
        emask = np.zeros((128, tot), np.float32)
        for w in range(NWIN):
            es = buckets[c][w]
            s0 = int(Soff[w])
            for i, e in enumerate(es):
                isrc[s0 + i] = src_pad[e]
                idst[s0 + i] = dst[e] - c * NLOC
                n = (dst[e] - c * NLOC) - w * 128
                emask[i % 128, s0 + (i // 128) * 128 + n] = 1.0
        pmask = np.zeros((128, NWIN * N_GRAPHS), np.float32)
        bl = batch[c * NLOC:(c + 1) * NLOC]
        for nl in range(NLOC):
            pmask[nl % 128, (nl // 128) * N_GRAPHS + bl[nl]] = 1.0

        m = dict(shared)
        m["xT"] = xT
        m["isrc"] = _wrap_idx(isrc)
        m["idst"] = _wrap_idx(idst)
        m["emask"] = _bf16(emask)
        m["pmask"] = _bf16(pmask)
        in_maps.append(m)
    return Tw, in_maps


def _build(Tw):
    Tw = tuple(Tw)
    TMAX = max(Tw)
    Soff = [0]
    for t in Tw:
        Soff.append(Soff[-1] + t * 128)
    TOT = Soff[-1]
    nc = bacc.Bacc("TRN2", target_bir_lowering=False, debug=False,
                   num_devices=NCORES)
    dt = mybir.dt
    AF = mybir.ActivationFunctionType
    OP = mybir.AluOpType

    def inp(name, shape, d):
        return nc.dram_tensor(name, shape, d, kind="ExternalInput")

    xT_in = inp("xT", [128, (KIN_PAD // 128) * NPAD], dt.bfloat16)
    isrc_in = inp("isrc", [128, TOT // 16], dt.int16)
    idst_in = inp("idst", [128, TOT // 16], dt.int16)
    emask_in = inp("emask", [128, TOT], dt.bfloat16)
    pmask_in = inp("pmask", [128, NWIN * N_GRAPHS], dt.bfloat16)
    w_in = [(inp("w1l", [128, 3 * 1024], dt.bfloat16),
             inp("w1r", [128, 3 * 1024], dt.bfloat16)),
            (inp("w2l", [128, 8 * 1024], dt.bfloat16),
             inp("w2r", [128, 8 * 1024], dt.bfloat16)),
            (inp("w3l", [128, 8 * 1536], dt.bfloat16),
             inp("w3r", [128, 8 * 1536], dt.bfloat16))]
    att_in = [inp("att1", [128, 1024], dt.bfloat16),
              inp("att2", [128, 1024], dt.bfloat16),
              inp("att3", [128, 1536], dt.bfloat16)]
    b_in = [inp("b1", [128, 1024], dt.float32),
            inp("b2", [128, 1024], dt.float32),
            inp("b3", [128, 256], dt.float32)]
    rcnt_in = inp("rcnt", [128, N_GRAPHS], dt.float32)
    wfc1_in = inp("wfc1", [128, 2 * 256], dt.bfloat16)
    wfc2_in = inp("wfc2", [128, 2 * 768], dt.bfloat16)
    bfc1_in = inp("bfc1", [128, 2], dt.float32)
    bfc2_in = inp("bfc2", [128, 768], dt.float32)
    shifts_in = inp("shifts", [128, 4], dt.float32)
    out_ext = nc.dram_tensor("out", [N_GRAPHS, NOUT], dt.float32,
                             kind="ExternalOutput")
    DEBUG = False
    if DEBUG:
        dbg_xl0 = nc.dram_tensor("dbg_xl0", [NPAD, 1024], dt.bfloat16,
                                 kind="ExternalOutput")
        dbg_xr0 = nc.dram_tensor("dbg_xr0", [NPAD, 1024], dt.bfloat16,
                                 kind="ExternalOutput")
        dbg_hd0 = nc.dram_tensor("dbg_hd0", [NPAD, 1024], dt.bfloat16,
                                 kind="ExternalOutput")
        dbg_hd1 = nc.dram_tensor("dbg_hd1", [NPAD, 1024], dt.bfloat16,
                                 kind="ExternalOutput")
        dbg_pool = nc.dram_tensor("dbg_pool", [256, N_GRAPHS], dt.float32,
                                  kind="ExternalOutput")
        dbg_xlf = nc.dram_tensor("dbg_xlf", [NPAD, 1024], dt.bfloat16,
                                 kind="ExternalOutput")
        dbg_s = nc.dram_tensor("dbg_s", [128, 1024], dt.bfloat16,
                               kind="ExternalOutput")
        dbg_sc = nc.dram_tensor("dbg_sc", [128, 4], dt.float32,
                                kind="ExternalOutput")
        dbg_den = nc.dram_tensor("dbg_den", [128, 4], dt.float32,
                                 kind="ExternalOutput")
        dbg_hn = nc.dram_tensor("dbg_hn", [128, 1024], dt.float32,
                                kind="ExternalOutput")
        dbg_num = nc.dram_tensor("dbg_num", [128, 1024], dt.float32,
                                 kind="ExternalOutput")
        dbg_rec = nc.dram_tensor("dbg_rec", [128, 4], dt.float32,
                                 kind="ExternalOutput")
        dbg_xsh = nc.dram_tensor("dbg_xsh", [128, 256], dt.bfloat16,
                                 kind="ExternalOutput")

    # internal DRAM
    xl_loc = [nc.dram_tensor(f"xl_loc{l}", [NPAD, F], dt.bfloat16)
              for l, (_, F, _, _) in enumerate(LAYERS)]
    xr_loc = [nc.dram_tensor(f"xr_loc{l}", [NPAD, F], dt.bfloat16)
              for l, (_, F, _, _) in enumerate(LAYERS)]
    xl_full = [nc.dram_tensor(f"xl_full{l}", [NCORES * NPAD, F], dt.bfloat16,
                              addr_space="Shared")
               for l, (_, F, _, _) in enumerate(LAYERS)]
    h_dram = [nc.dram_tensor(f"h_dram{l}", [NPAD, 1024], dt.bfloat16)
              for l in range(2)]
    pool_loc = nc.dram_tensor("pool_loc", [256, N_GRAPHS], dt.float32)
    pool_full = nc.dram_tensor("pool_full", [256, N_GRAPHS], dt.float32,
                               addr_space="Shared")

    rg = [list(range(NCORES))]

    with tile.TileContext(nc) as tc:
        with (
            tc.tile_pool(name="persist", bufs=1) as ppool,
            tc.tile_pool(name="psPool", bufs=1, space="PSUM") as psPool,
        ):
            isrc_t = ppool.tile([128, TOT // 16], dt.int16)
            nc.sync.dma_start(out=isrc_t[:, :], in_=isrc_in[:, :])
            idst_t = ppool.tile([128, TOT // 16], dt.int16)
            nc.sync.dma_start(out=idst_t[:, :], in_=idst_in[:, :])
            shifts_t = ppool.tile([128, 4], dt.float32)
            nc.sync.dma_start(out=shifts_t[:, :], in_=shifts_in[:, :])

            pool_ps = [None, None]
            hT_cur = None

            for l, (K, F, H, concat) in enumerate(LAYERS):
                KB = K // 128
                NCH = F // 512          # 512-col chunks of F

                with (
                    tc.tile_pool(name=f"w{l}", bufs=1) as wpool,
                    tc.tile_pool(name=f"hT{l}", bufs=1) as hpool,
                    tc.tile_pool(name=f"mm{l}", bufs=4) as mmpool,
                    tc.tile_pool(name=f"psA{l}", bufs=2,
                                 space="PSUM") as psA,
                ):
                    # ---- load hT (layer input, [128, KB, NPAD] bf16) ----
                    if l == 0:
                        hT = hpool.tile([128, KB, NPAD], dt.bfloat16, tag="hT")
                        for b in range(KB):
                            nc.sync.dma_start(
                                out=hT[:, b, :],
                                in_=xT_in[:, b * NPAD:(b + 1) * NPAD])
                    else:
                        hT = hpool.tile([128, KB, NPAD], dt.bfloat16, tag="hT")
                        for b in range(KB):
                            nc.sync.dma_start(
                                out=hT[:, b, :],
                                in_=h_dram[l - 1][:, b * 128:(b + 1) * 128],
                                transpose=True)

                    # ---- weights ----
                    wl_t = wpool.tile([128, KB, F], dt.bfloat16)
                    wr_t = wpool.tile([128, KB, F], dt.bfloat16)
                    for wt, win in ((wl_t, w_in[l][0]), (wr_t, w_in[l][1])):
                        for b in range(KB):
                            nc.sync.dma_start(
                                out=wt[:, b, :],
                                in_=win[:, b * F:(b + 1) * F])

                    # ---- A: projections xl = h @ wl.T, xr = h @ wr.T ----
                    for side, (wt, dst_dram) in enumerate(
                            ((wl_t, xl_loc[l]), (wr_t, xr_loc[l]))):
                        if side == 1:
                            # AllGather xl overlaps with the xr projection
                            nc.gpsimd.collective_compute(
                                "AllGather", mybir.AluOpType.bypass,
                                replica_groups=rg,
                                ins=[xl_loc[l].ap().opt()],
                                outs=[xl_full[l].ap().opt()])
                        for t in range(NWIN):
                            for ch in range(NCH):
                                ps = psA.tile([128, 512], dt.float32,
                                              tag="mmps")
                                for b in range(KB):
                                    nc.tensor.matmul(
                                        ps[:, :],
                                        hT[:, b, t * 128:(t + 1) * 128],
                                        wt[:, b, ch * 512:(ch + 1) * 512],
                                        start=(b == 0), stop=(b == KB - 1))
                                ob = mmpool.tile([128, 512], dt.bfloat16,
                                                 tag="mmout")
                                nc.scalar.copy(ob[:, :], ps[:, :])
                                nc.sync.dma_start(
                                    out=dst_dram[t * 128:(t + 1) * 128,
                                                 ch * 512:(ch + 1) * 512],
                                    in_=ob[:, :])

                # ---- C: edge phase ----
                gbufs = 3 if F <= 1024 else 2
                with (
                    tc.tile_pool(name=f"g{l}", bufs=gbufs) as gpool,
                    tc.tile_pool(name=f"ew{l}", bufs=3) as epool,
                    tc.tile_pool(name=f"es{l}", bufs=3) as spool,
                    tc.tile_pool(name=f"psE{l}", bufs=1,
                                 space="PSUM") as psE,
                    tc.tile_pool(name=f"psD{l}", bufs=2,
                                 space="PSUM") as psD,
                    tc.tile_pool(name=f"aux{l}", bufs=1) as auxpool,
                ):
                    att_t = auxpool.tile([128, F], dt.bfloat16)
                    nc.sync.dma_start(out=att_t[:, :], in_=att_in[l][:, :])
                    bias_t = auxpool.tile([128, F if concat else 256],
                                          dt.float32)
                    nc.sync.dma_start(out=bias_t[:, :], in_=b_in[l][:, :])
                    if l == 2:
                        pmask_t = auxpool.tile([128, NWIN * N_GRAPHS],
                                               dt.bfloat16)
                        nc.sync.dma_start(out=pmask_t[:, :], in_=pmask_in[:, :])
                        pool_ps[0] = psPool.tile([128, N_GRAPHS], dt.float32,
                                                 tag="poolps0", name="poolps0")
                        pool_ps[1] = psPool.tile([128, N_GRAPHS], dt.float32,
                                                 tag="poolps1", name="poolps1")

                    for w in range(NWIN):
                        T = Tw[w]
                        S = T * 128
                        mask_t = epool.tile([128, TMAX * 128], dt.bfloat16,
                                            tag="emask")
                        nc.sync.dma_start(
                            out=mask_t[:, :S],
                            in_=emask_in[:, Soff[w]:Soff[w + 1]])
                        gx = gpool.tile([128, TMAX, F], dt.bfloat16, tag="gx")
                        nc.gpsimd.dma_gather(
                            gx[:, :T, :], xl_full[l][:, :],
                            isrc_t[:, Soff[w] // 16:Soff[w + 1] // 16],
                            num_idxs=S, num_idxs_reg=S, elem_size=F)
                        gr = gpool.tile([128, TMAX, F], dt.bfloat16, tag="gr")
                        nc.gpsimd.dma_gather(
                            gr[:, :T, :], xr_loc[l][:, :],
                            idst_t[:, Soff[w] // 16:Soff[w + 1] // 16],
                            num_idxs=S, num_idxs_reg=S, elem_size=F)

                        # pass 0: scores -> ex for all tiles
                        ex_w = spool.tile([128, TMAX, H], dt.float32,
                                          tag="exw")
                        exb_w = spool.tile([128, TMAX, H], dt.bfloat16,
                                           tag="exbw")
                        for t in range(T):
                            s_t = spool.tile([128, F], dt.bfloat16, tag="s")
                            nc.vector.tensor_tensor(
                                s_t[:, :], gx[:, t, :], gr[:, t, :], OP.add)
                            nc.scalar.activation(
                                s_t[:, :], s_t[:, :], AF.Prelu, alpha=SLOPE)
                            nc.vector.tensor_tensor(
                                s_t[:, :], s_t[:, :], att_t[:, :], OP.mult)
                            sc_t = spool.tile([128, H], dt.float32, tag="sc")
                            nc.vector.tensor_reduce(
                                sc_t[:, :],
                                s_t[:, :].rearrange("p (h c) -> p h c", h=H),
                                mybir.AxisListType.X, OP.add)
                            nc.scalar.activation(
                                ex_w[:, t, :], sc_t[:, :], AF.Exp,
                                bias=shifts_t[:, l:l + 1], scale=1.0)
                        nc.scalar.copy(exb_w[:, :T, :], ex_w[:, :T, :])
                        # scaling pass on ACT: gx[:, t, hslice] *= ex[:, h]
                        for t in range(T):
                            for h in range(H):
                                nc.scalar.activation(
                                    gx[:, t, h * 256:(h + 1) * 256],
                                    gx[:, t, h * 256:(h + 1) * 256],
                                    AF.Copy, scale=ex_w[:, t, h:h + 1])

                        # denominator (own psum tile, contiguous group)
                        ps_den = psD.tile([128, H], dt.float32, tag="den")
                        for t in range(T):
                            nc.tensor.matmul(
                                ps_den[:, :], mask_t[:, t * 128:(t + 1) * 128],
                                exb_w[:, t, :], start=(t == 0),
                                stop=(t == T - 1))
                        den_t = spool.tile([128, H], dt.float32, tag="wden")
                        nc.vector.tensor_scalar(den_t[:, :], ps_den[:, :H],
                                                float(EPS), None, OP.add)
                        rec_t = spool.tile([128, H], dt.float32, tag="wrec")
                        nc.vector.reciprocal(rec_t[:, :], den_t[:, :])

                        # aggregation in head groups; one psum tile per head
                        if concat:
                            hn = spool.tile([128, F], dt.float32, tag="hn")
                        else:
                            acc = spool.tile([128, 256], dt.float32, tag="acc")
                        groups = ([tuple(range(H))] if H <= 4
                                  else [(0, 1, 2), (3, 4, 5)])
                        for grp in groups:
                            ps_g = [psE.tile([128, 256], dt.float32,
                                             tag=f"agg{j}", name=f"agg{j}")
                                    for j in range(len(grp))]
                            for t in range(T):
                                for j, h in enumerate(grp):
                                    nc.tensor.matmul(
                                        ps_g[j][:, :],
                                        mask_t[:, t * 128:(t + 1) * 128],
                                        gx[:, t, h * 256:(h + 1) * 256],
                                        start=(t == 0), stop=(t == T - 1))
                            for j, h in enumerate(grp):
                                if concat:
                                    nc.vector.tensor_scalar(
                                        hn[:, h * 256:(h + 1) * 256],
                                        ps_g[j][:, :], rec_t[:, h:h + 1],
                                        None, OP.mult)
                                elif h == 0:
                                    nc.vector.tensor_scalar(
                                        acc[:, :], ps_g[j][:, :],
                                        rec_t[:, 0:1], None, OP.mult)
                                else:
                                    nc.vector.scalar_tensor_tensor(
                                        acc[:, :], ps_g[j][:, :],
                                        rec_t[:, h:h + 1], acc[:, :],
                                        OP.mult, OP.add)

                        # ---- window epilogue ----
                        if concat:
                            nc.vector.tensor_tensor(hn[:, :], hn[:, :],
                                                    bias_t[:, :], OP.add)
                            # elu: max(x, exp(min(x,0)) - 1)
                            mm = spool.tile([128, F], dt.float32, tag="elu_m")
                            nc.vector.tensor_scalar(mm[:, :], hn[:, :], 0.0,
                                                    None, OP.min)
                            nc.scalar.activation(mm[:, :], mm[:, :], AF.Exp)
                            nc.vector.tensor_scalar(mm[:, :], mm[:, :], -1.0,
                                                    None, OP.add)
                            hb = spool.tile([128, F], dt.bfloat16, tag="hb")
                            nc.vector.tensor_tensor(hb[:, :], hn[:, :],
                                                    mm[:, :], OP.max)
                            nc.sync.dma_start(
                                out=h_dram[l][w * 128:(w + 1) * 128, :],
                                in_=hb[:, :])
                        else:
                            nc.vector.tensor_scalar(acc[:, :], acc[:, :],
                                                    1.0 / H, None, OP.mult)
                            nc.vector.tensor_tensor(acc[:, :], acc[:, :],
                                                    bias_t[:, :], OP.add)
                            # l2 normalize rows
                            ss = spool.tile([128, 1], dt.float32, tag="ss")
                            trash2 = spool.tile([128, 256], dt.float32,
                                                tag="trash2")
                            nc.vector.scalar_tensor_tensor(
                                trash2[:, :], acc[:, :], 1.0, acc[:, :],
                                OP.mult, OP.mult, accum_out=ss[:, :])
                            nrm = spool.tile([128, 1], dt.float32, tag="nrm")
                            nc.scalar.activation(nrm[:, :], ss[:, :], AF.Sqrt)
                            nc.vector.tensor_scalar(nrm[:, :], nrm[:, :],
                                                    1e-12, None, OP.max)
                            rn = spool.tile([128, 1], dt.float32, tag="rn")
                            nc.vector.reciprocal(rn[:, :], nrm[:, :])
                            hb = spool.tile([128, 256], dt.bfloat16,
                                            tag="hb")
                            nc.vector.tensor_scalar(hb[:, :], acc[:, :],
                                                    rn[:, :], None, OP.mult)
                            # pool: pooled_T[c, g] += sum_n h[n, c] pmask[n, g]
                            for b in range(2):
                                nc.tensor.matmul(
                                    pool_ps[b][:, :],
                                    hb[:, b * 128:(b + 1) * 128],
                                    pmask_t[:, w * N_GRAPHS:
                                            (w + 1) * N_GRAPHS],
                                    start=(w == 0), stop=(w == NWIN - 1))

            if DEBUG:
                nc.sync.dma_start(out=dbg_xl0[:, :], in_=xl_loc[0][:, :])
                nc.sync.dma_start(out=dbg_xr0[:, :], in_=xr_loc[0][:, :])
                nc.sync.dma_start(out=dbg_hd0[:, :], in_=h_dram[0][:, :])
                nc.sync.dma_start(out=dbg_hd1[:, :], in_=h_dram[1][:, :])
                nc.sync.dma_start(out=dbg_xlf[:, :],
                                  in_=xl_full[0][5 * NPAD:6 * NPAD, :])

            # ---- D: pooled -> AllReduce -> MLP ----
            with (
                tc.tile_pool(name="mlp", bufs=1) as mpool,
                tc.tile_pool(name="psM", bufs=1, space="PSUM") as psM,
            ):
                for b in range(2):
                    pl = mpool.tile([128, N_GRAPHS], dt.float32)
                    nc.vector.tensor_copy(pl[:, :], pool_ps[b][:, :])
                    nc.sync.dma_start(
                        out=pool_loc[b * 128:(b + 1) * 128, :], in_=pl[:, :])
                nc.gpsimd.collective_compute(
                    "AllReduce", mybir.AluOpType.add, replica_groups=rg,
                    ins=[pool_loc.ap().opt()],
                    outs=[pool_full.ap().opt()])
                if DEBUG:
                    nc.sync.dma_start(out=dbg_pool[:, :], in_=pool_full[:, :])

                rcnt_t = mpool.tile([128, N_GRAPHS], dt.float32)
                nc.sync.dma_start(out=rcnt_t[:, :], in_=rcnt_in[:, :])
                pz = mpool.tile([128, 2, N_GRAPHS], dt.bfloat16)
                for b in range(2):
                    pf = mpool.tile([128, N_GRAPHS], dt.float32, tag="pf")
                    nc.sync.dma_start(out=pf[:, :],
                                      in_=pool_full[b * 128:(b + 1) * 128, :])
                    nc.vector.tensor_tensor(pz[:, b, :], pf[:, :],
                                            rcnt_t[:, :], OP.mult)

                wfc1_t = mpool.tile([128, 2, 256], dt.bfloat16)
                wfc2_t = mpool.tile([128, 2, 768], dt.bfloat16)
                for b in range(2):
                    nc.sync.dma_start(out=wfc1_t[:, b, :],
                                      in_=wfc1_in[:, b * 256:(b + 1) * 256])
                    nc.sync.dma_start(out=wfc2_t[:, b, :],
                                      in_=wfc2_in[:, b * 768:(b + 1) * 768])
                bfc1_t = mpool.tile([128, 2], dt.float32)
                nc.sync.dma_start(out=bfc1_t[:, :], in_=bfc1_in[:, :])
                bfc2_t = mpool.tile([128, 768], dt.float32)
                nc.sync.dma_start(out=bfc2_t[:, :], in_=bfc2_in[:, :])

                z1 = mpool.tile([128, 2, N_GRAPHS], dt.bfloat16)
                for it in range(2):
                    ps1 = psM.tile([128, N_GRAPHS], dt.float32, tag="ps1")
                    for b in range(2):
                        nc.tensor.matmul(
                            ps1[:, :],
                            wfc1_t[:, b, it * 128:(it + 1) * 128],
                            pz[:, b, :], start=(b == 0), stop=(b == 1))
                    nc.scalar.activation(z1[:, it, :], ps1[:, :], AF.Relu,
                                         bias=bfc1_t[:, it:it + 1], scale=1.0)

                for gt in range(N_GRAPHS // 128):
                    ps2 = psM.tile([128, 768], dt.float32, tag="ps2")
                    for jc, (j0, jw) in enumerate(((0, 512), (512, 256))):
                        for b in range(2):
                            nc.tensor.matmul(
                                ps2[:, j0:j0 + jw],
                                z1[:, b, gt * 128:(gt + 1) * 128],
                                wfc2_t[:, b, j0:j0 + jw],
                                start=(b == 0), stop=(b == 1))
                    zo = mpool.tile([128, 768], dt.float32, tag="zo")
                    nc.vector.tensor_tensor(zo[:, :], ps2[:, :],
                                            bfc2_t[:, :], OP.add)
                    nc.sync.dma_start(
                        out=out_ext[gt * 128:(gt + 1) * 128, :], in_=zo[:, :])

    nc.compile()
    return nc


def kernel(**inputs):
    T, in_maps = _preprocess(inputs)
    if T not in _PROG_CACHE:
        _PROG_CACHE[T] = _build(T)
    nc = _PROG_CACHE[T]
    r = run_bass_kernel_spmd(nc, in_maps, list(range(NCORES)), trace=False)
    return r.results[0]["out"]



# revision 26
# speedup vs baseline: 1.1159x; 1.1159x over previous
"""GATv2 (3-layer, PyG semantics) + global mean pool + MLP on 8 trn2 NeuronCores.

Self-contained: hardcodes problem shapes from nn_GAT_47906065219807.
Sharding: data-parallel over contiguous node ranges (2500 nodes/core); each
core owns edges whose dst lands in its range (edges cross shards via an
AllGather of the source-side projections).

v2 edge phase:
  - layers 0/1: gr is expanded from the xr window tile by a PE matmul with a
    transposed edge mask, and gx is added in-PSUM via an identity matmul;
    ACT applies Prelu straight from PSUM (no DVE add, no gr gather).
  - softmax weights are folded into the aggregation matmul's stationary mask
    (alphaM = emask * ex) built by one broadcast DVE op per tile; the
    division by the softmax denominator happens per-node after aggregation.
  - att-dot via fused tensor_tensor_reduce per head.
"""
import numpy as np
import ml_dtypes

import concourse.bacc as bacc
import concourse.mybir as mybir
import concourse.tile as tile
from concourse.bass_utils import run_bass_kernel_spmd

# problem constants
N_NODES = 20000
N_EDGES = 120000
N_GRAPHS = 512
F_IN = 300
NHID = 256
NOUT = 768
SLOPE = 0.2
EPS = 1e-16

NCORES = 8
NLOC = N_NODES // NCORES            # 2500
NPAD = 2560                         # 20 tiles of 128
NWIN = NPAD // 128                  # 20 windows / node tiles per core
KIN_PAD = 384                       # F_IN padded to 3*128

# per-layer dims: (K_in_padded, F_out, heads, concat)
LAYERS = [
    (KIN_PAD, 4 * NHID, 4, True),
    (4 * NHID, 4 * NHID, 4, True),
    (4 * NHID, 6 * NHID, 6, False),
]

_BF = ml_dtypes.bfloat16
_PROG_CACHE = {}
SIM_SAFE = False      # sim lacks the fused DVE-accumulator writeback
STT_SCORES = False    # fused per-head stt+accum scores (else mult+3D-reduce)
DEBUG = False


def _bf16(a):
    return np.ascontiguousarray(a.astype(_BF)).view(np.uint16)


def _wrap_idx(flat_idx):
    """dma_gather index layout: slot i -> [i % 16, i // 16], replicated to
    128 partitions (8 Q7 cores x 16)."""
    n = flat_idx.shape[0]
    assert n % 16 == 0
    w = flat_idx.reshape(n // 16, 16).T.astype(np.int16)
    return np.tile(w, (8, 1)).copy()


def _preprocess(inputs):
    """Host-side sharding/layout. Returns (T, in_maps)."""
    x = np.asarray(inputs["x"], np.float32)
    ei = np.asarray(inputs["edge_index"]).astype(np.int64)
    batch = np.asarray(inputs["batch"]).astype(np.int64)

    loops = np.arange(N_NODES, dtype=np.int64)
    src = np.concatenate([ei[0], loops])
    dst = np.concatenate([ei[1], loops])

    # padded-global row index into the AllGather output [NCORES*NPAD, F]
    src_pad = (src // NLOC) * NPAD + (src % NLOC)

    core_of = dst // NLOC
    # per (core, window) edge buckets
    buckets = [[[] for _ in range(NWIN)] for _ in range(NCORES)]
    order = np.argsort(dst, kind="stable")
    for e in order:
        c = core_of[e]
        dl = dst[e] - c * NLOC
        buckets[c][dl // 128].append(e)
    Tw = tuple(
        max((len(buckets[c][w]) + 127) // 128 for c in range(NCORES))
        for w in range(NWIN)
    )
    Soff = np.concatenate([[0], np.cumsum([t * 128 for t in Tw])])

    # graph counts -> reciprocal (reference divides by max(cnt,1))
    cnt = np.bincount(batch, minlength=N_GRAPHS).astype(np.float32)
    rcnt = 1.0 / np.maximum(cnt, 1.0)

    # shared (per-core identical) tensors
    def wT_pad(w, kpad):
        # host layout [128, KB, F]: [p, b, f] = w[f-major...]  w is [F, K]
        wt = w.T.astype(np.float32)                    # [K, F]
        K, F = wt.shape
        out = np.zeros((kpad, F), np.float32)
        out[:K] = wt
        return _bf16(out.reshape(kpad // 128, 128, F).transpose(1, 0, 2)
                     .reshape(128, (kpad // 128) * F))

    shared = {
        "w1l": wT_pad(np.asarray(inputs["c1_wl"]), KIN_PAD),
        "w1r": wT_pad(np.asarray(inputs["c1_wr"]), KIN_PAD),
        "w2l": wT_pad(np.asarray(inputs["c2_wl"]), 4 * NHID),
        "w2r": wT_pad(np.asarray(inputs["c2_wr"]), 4 * NHID),
        "w3l": wT_pad(np.asarray(inputs["c3_wl"]), 4 * NHID),
        "w3r": wT_pad(np.asarray(inputs["c3_wr"]), 4 * NHID),
        "att1": _bf16(np.tile(np.asarray(inputs["c1_att"]).reshape(1, -1), (128, 1))),
        "att2": _bf16(np.tile(np.asarray(inputs["c2_att"]).reshape(1, -1), (128, 1))),
        "att3": _bf16(np.tile(np.asarray(inputs["c3_att"]).reshape(1, -1), (128, 1))),
        "b1": np.tile(np.asarray(inputs["c1_b"], np.float32).reshape(1, -1), (128, 1)),
        "b2": np.tile(np.asarray(inputs["c2_b"], np.float32).reshape(1, -1), (128, 1)),
        "b3": np.tile(np.asarray(inputs["c3_b"], np.float32).reshape(1, -1), (128, 1)),
        "rcnt": np.tile(rcnt.reshape(1, -1), (128, 1)).astype(np.float32),
        "wfc1": wT_pad(np.asarray(inputs["fp1_w"]), 256),
        "wfc2": wT_pad(np.asarray(inputs["fp2_w"]), 256),
        "bfc1": np.asarray(inputs["fp1_b"], np.float32).reshape(2, 128).T.copy(),
        "bfc2": np.tile(np.asarray(inputs["fp2_b"], np.float32).reshape(1, -1),
                        (128, 1)),
        "ident": _bf16(np.eye(128, dtype=np.float32)),
    }

    in_maps = []
    for c in range(NCORES):
        xc = np.zeros((NPAD, KIN_PAD), np.float32)
        xc[:NLOC, :F_IN] = x[c * NLOC:(c + 1) * NLOC]
        # xT host layout [128, 3, NPAD]
        xT = xc.T.reshape(KIN_PAD // 128, 128, NPAD).transpose(1, 0, 2)
        xT = _bf16(xT.reshape(128, (KIN_PAD // 128) * NPAD))

        tot = int(Soff[-1])
        isrc = np.zeros(tot, np.int64)
        idst = np.zeros(tot, np.int64)
        emask = np.zeros((128, tot), np.float32)
        emaskT = np.zeros((128, tot), np.float32)
        for w in range(NWIN):
            es = buckets[c][w]
            s0 = int(Soff[w])
            for i, e in enumerate(es):
                isrc[s0 + i] = src_pad[e]
                idst[s0 + i] = dst[e] - c * NLOC
                n = (dst[e] - c * NLOC) - w * 128
                emask[i % 128, s0 + (i // 128) * 128 + n] = 1.0
                emaskT[n, s0 + (i // 128) * 128 + (i % 128)] = 1.0
        pmask = np.zeros((128, NWIN * N_GRAPHS), np.float32)
        bl = batch[c * NLOC:(c + 1) * NLOC]
        for nl in range(NLOC):
            pmask[nl % 128, (nl // 128) * N_GRAPHS + bl[nl]] = 1.0

        m = dict(shared)
        m["xT"] = xT
        m["isrc"] = _wrap_idx(isrc)
        m["idst"] = _wrap_idx(idst)
        m["emask"] = _bf16(emask)
        m["emaskT"] = _bf16(emaskT)
        m["pmask"] = _bf16(pmask)
        in_maps.append(m)
    return Tw, in_maps


def _build(Tw):
    Tw = tuple(Tw)
    TMAX = max(Tw)
    Soff = [0]
    for t in Tw:
        Soff.append(Soff[-1] + t * 128)
    TOT = Soff[-1]
    nc = bacc.Bacc("TRN2", target_bir_lowering=False, debug=False,
                   num_devices=NCORES)
    dt = mybir.dt
    AF = mybir.ActivationFunctionType
    OP = mybir.AluOpType

    def inp(name, shape, d):
        return nc.dram_tensor(name, shape, d, kind="ExternalInput")

    xT_in = inp("xT", [128, (KIN_PAD // 128) * NPAD], dt.bfloat16)
    isrc_in = inp("isrc", [128, TOT // 16], dt.int16)
    idst_in = inp("idst", [128, TOT // 16], dt.int16)
    emask_in = inp("emask", [128, TOT], dt.bfloat16)
    emaskT_in = inp("emaskT", [128, TOT], dt.bfloat16)
    pmask_in = inp("pmask", [128, NWIN * N_GRAPHS], dt.bfloat16)
    ident_in = inp("ident", [128, 128], dt.bfloat16)
    w_in = [(inp("w1l", [128, 3 * 1024], dt.bfloat16),
             inp("w1r", [128, 3 * 1024], dt.bfloat16)),
            (inp("w2l", [128, 8 * 1024], dt.bfloat16),
             inp("w2r", [128, 8 * 1024], dt.bfloat16)),
            (inp("w3l", [128, 8 * 1536], dt.bfloat16),
             inp("w3r", [128, 8 * 1536], dt.bfloat16))]
    att_in = [inp("att1", [128, 1024], dt.bfloat16),
              inp("att2", [128, 1024], dt.bfloat16),
              inp("att3", [128, 1536], dt.bfloat16)]
    b_in = [inp("b1", [128, 1024], dt.float32),
            inp("b2", [128, 1024], dt.float32),
            inp("b3", [128, 256], dt.float32)]
    rcnt_in = inp("rcnt", [128, N_GRAPHS], dt.float32)
    wfc1_in = inp("wfc1", [128, 2 * 256], dt.bfloat16)
    wfc2_in = inp("wfc2", [128, 2 * 768], dt.bfloat16)
    bfc1_in = inp("bfc1", [128, 2], dt.float32)
    bfc2_in = inp("bfc2", [128, 768], dt.float32)
    out_ext = nc.dram_tensor("out", [N_GRAPHS, NOUT], dt.float32,
                             kind="ExternalOutput")
    if DEBUG:
        dbg_z = nc.dram_tensor("dbg_z", [128, 1024], dt.float32,
                               kind="ExternalOutput")
        dbg_s = nc.dram_tensor("dbg_s", [128, 1024], dt.bfloat16,
                               kind="ExternalOutput")
        dbg_sc = nc.dram_tensor("dbg_sc", [128, 4], dt.float32,
                                kind="ExternalOutput")
        dbg_gx = nc.dram_tensor("dbg_gx", [128, 1024], dt.bfloat16,
                                kind="ExternalOutput")
        dbg_xr = nc.dram_tensor("dbg_xr", [128, 1024], dt.bfloat16,
                                kind="ExternalOutput")
        dbg_mT = nc.dram_tensor("dbg_mT", [128, 128], dt.bfloat16,
                                kind="ExternalOutput")

    # internal DRAM
    xl_loc = [nc.dram_tensor(f"xl_loc{l}", [NPAD, F], dt.bfloat16)
              for l, (_, F, _, _) in enumerate(LAYERS)]
    xr_loc2 = nc.dram_tensor("xr_loc2", [NPAD, 1536], dt.bfloat16)
    xl_full = [nc.dram_tensor(f"xl_full{l}", [NCORES * NPAD, F], dt.bfloat16,
                              addr_space="Shared")
               for l, (_, F, _, _) in enumerate(LAYERS)]
    h_dram = [nc.dram_tensor(f"h_dram{l}", [NPAD, 1024], dt.bfloat16)
              for l in range(2)]
    pool_loc = nc.dram_tensor("pool_loc", [256, N_GRAPHS], dt.float32)
    pool_full = nc.dram_tensor("pool_full", [256, N_GRAPHS], dt.float32,
                               addr_space="Shared")

    rg = [list(range(NCORES))]

    with tile.TileContext(nc) as tc:
        with (
            tc.tile_pool(name="persist", bufs=1) as ppool,
        ):
            isrc_t = ppool.tile([128, TOT // 16], dt.int16)
            nc.sync.dma_start(out=isrc_t[:, :], in_=isrc_in[:, :])
            idst_t = ppool.tile([128, TOT // 16], dt.int16)
            nc.sync.dma_start(out=idst_t[:, :], in_=idst_in[:, :])
            ident_t = ppool.tile([128, 128], dt.bfloat16)
            nc.sync.dma_start(out=ident_t[:, :], in_=ident_in[:, :])

            pool_ps = [None, None]

            for l, (K, F, H, concat) in enumerate(LAYERS):
                KB = K // 128
                NCH = F // 512          # 512-col chunks of F
                expand = l < 2          # gr via PE mask-expand + in-PSUM add

                with tc.tile_pool(name=f"xr{l}", bufs=1) as xrpool:
                    if expand:
                        xr_sb = xrpool.tile([128, NWIN, F], dt.bfloat16)

                    with (
                        tc.tile_pool(name=f"w{l}", bufs=1) as wpool,
                        tc.tile_pool(name=f"hT{l}", bufs=1) as hpool,
                        tc.tile_pool(name=f"mm{l}", bufs=4) as mmpool,
                        tc.tile_pool(name=f"psA{l}", bufs=2,
                                     space="PSUM") as psA,
                    ):
                        # ---- load hT (layer input, [128, KB, NPAD] bf16) ----
                        hT = hpool.tile([128, KB, NPAD], dt.bfloat16, tag="hT")
                        if l == 0:
                            for b in range(KB):
                                nc.sync.dma_start(
                                    out=hT[:, b, :],
                                    in_=xT_in[:, b * NPAD:(b + 1) * NPAD])
                        else:
                            for b in range(KB):
                                nc.sync.dma_start(
                                    out=hT[:, b, :],
                                    in_=h_dram[l - 1][:, b * 128:(b + 1) * 128],
                                    transpose=True)

                        # ---- weights ----
                        wl_t = wpool.tile([128, KB, F], dt.bfloat16)
                        wr_t = wpool.tile([128, KB, F], dt.bfloat16)
                        for wt, win in ((wl_t, w_in[l][0]), (wr_t, w_in[l][1])):
                            for b in range(KB):
                                nc.sync.dma_start(
                                    out=wt[:, b, :],
                                    in_=win[:, b * F:(b + 1) * F])

                        # ---- A: projections xl = h @ wl.T, xr = h @ wr.T ----
                        for side, wt in ((0, wl_t), (1, wr_t)):
                            if side == 1:
                                # AllGather xl overlaps with the xr projection
                                nc.gpsimd.collective_compute(
                                    "AllGather", mybir.AluOpType.bypass,
                                    replica_groups=rg,
                                    ins=[xl_loc[l].ap().opt()],
                                    outs=[xl_full[l].ap().opt()])
                            for t in range(NWIN):
                                for ch in range(NCH):
                                    ps = psA.tile([128, 512], dt.float32,
                                                  tag="mmps")
                                    for b in range(KB):
                                        nc.tensor.matmul(
                                            ps[:, :],
                                            hT[:, b, t * 128:(t + 1) * 128],
                                            wt[:, b, ch * 512:(ch + 1) * 512],
                                            start=(b == 0), stop=(b == KB - 1))
                                    if side == 1 and expand:
                                        nc.scalar.copy(
                                            xr_sb[:, t, ch * 512:(ch + 1) * 512],
                                            ps[:, :])
                                    else:
                                        ob = mmpool.tile([128, 512], dt.bfloat16,
                                                         tag="mmout")
                                        nc.scalar.copy(ob[:, :], ps[:, :])
                                        dst_dram = (xl_loc[l] if side == 0
                                                    else xr_loc2)
                                        nc.sync.dma_start(
                                            out=dst_dram[t * 128:(t + 1) * 128,
                                                         ch * 512:(ch + 1) * 512],
                                            in_=ob[:, :])

                    # ---- C: edge phase ----
                    with (
                        tc.tile_pool(name=f"g{l}", bufs=3 if expand else 2) as gpool,
                        tc.tile_pool(name=f"ew{l}", bufs=2) as epool,
                        tc.tile_pool(name=f"es{l}", bufs=3) as spool,
                        tc.tile_pool(name=f"am{l}",
                                     bufs=(3 if l < 2 else TMAX + 1)) as ampool,
                        tc.tile_pool(name=f"psZ{l}", bufs=2,
                                     space="PSUM") as psZ,
                        tc.tile_pool(name=f"psE{l}", bufs=1,
                                     space="PSUM") as psE,
                        tc.tile_pool(name=f"psD{l}", bufs=1,
                                     space="PSUM") as psD,
                        tc.tile_pool(name=f"psP{l}", bufs=1,
                                     space="PSUM") as psPool,
                        tc.tile_pool(name=f"aux{l}", bufs=1) as auxpool,
                    ):
                        att_t = auxpool.tile([128, F], dt.bfloat16)
                        nc.sync.dma_start(out=att_t[:, :], in_=att_in[l][:, :])
                        bias_t = auxpool.tile([128, F if concat else 256],
                                              dt.float32)
                        nc.sync.dma_start(out=bias_t[:, :], in_=b_in[l][:, :])
                        if l == 2:
                            pmask_t = auxpool.tile([128, NWIN * N_GRAPHS],
                                                   dt.bfloat16)
                            nc.sync.dma_start(out=pmask_t[:, :],
                                              in_=pmask_in[:, :])
                            pool_ps[0] = psPool.tile([128, N_GRAPHS], dt.float32,
                                                     tag="poolps0",
                                                     name="poolps0")
                            pool_ps[1] = psPool.tile([128, N_GRAPHS], dt.float32,
                                                     tag="poolps1",
                                                     name="poolps1")

                        for w in range(NWIN):
                            T = Tw[w]
                            S = T * 128
                            mask_t = epool.tile([128, TMAX * 128], dt.bfloat16,
                                                tag="emask")
                            nc.sync.dma_start(
                                out=mask_t[:, :S],
                                in_=emask_in[:, Soff[w]:Soff[w + 1]])
                            gx = gpool.tile([128, TMAX, F], dt.bfloat16,
                                            tag="gx")
                            nc.gpsimd.dma_gather(
                                gx[:, :T, :], xl_full[l][:, :],
                                isrc_t[:, Soff[w] // 16:Soff[w + 1] // 16],
                                num_idxs=S, num_idxs_reg=S, elem_size=F)
                            if expand:
                                maskT_t = epool.tile([128, TMAX * 128],
                                                     dt.bfloat16, tag="emaskT")
                                nc.sync.dma_start(
                                    out=maskT_t[:, :S],
                                    in_=emaskT_in[:, Soff[w]:Soff[w + 1]])
                            else:
                                gr = gpool.tile([128, TMAX, F], dt.bfloat16,
                                                tag="gr")
                                nc.gpsimd.dma_gather(
                                    gr[:, :T, :], xr_loc2[:, :],
                                    idst_t[:, Soff[w] // 16:Soff[w + 1] // 16],
                                    num_idxs=S, num_idxs_reg=S, elem_size=F)

                            ex_w = spool.tile([128, TMAX, H], dt.float32,
                                              tag="exw")
                            exb_w = spool.tile([128, TMAX, H], dt.bfloat16,
                                               tag="exbw")
                            ps_den = psD.tile([128, H], dt.float32, tag="den")
                            NAGG = H if concat else H // 2
                            ps_g = [psE.tile([128, 256], dt.float32,
                                             tag=f"agg{j}", name=f"agg{j}")
                                    for j in range(NAGG)]

                            aM_w = []
                            for t in range(T):
                                # ---- s = prelu(xl[src] + xr[dst]) ----
                                s_t = spool.tile([128, F], dt.bfloat16, tag="s")
                                if expand:
                                    ps_z = None
                                    for ch in range(NCH):
                                        ps_zc = psZ.tile([128, 512],
                                                         dt.float32, tag="z")
                                        nc.tensor.matmul(
                                            ps_zc[:, :],
                                            maskT_t[:, t * 128:(t + 1) * 128],
                                            xr_sb[:, w, ch * 512:(ch + 1) * 512],
                                            start=True, stop=False)
                                        nc.tensor.matmul(
                                            ps_zc[:, :],
                                            ident_t[:, :],
                                            gx[:, t, ch * 512:(ch + 1) * 512],
                                            start=False, stop=True)
                                        nc.scalar.activation(
                                            s_t[:, ch * 512:(ch + 1) * 512],
                                            ps_zc[:, :], AF.Prelu,
                                            alpha=SLOPE)
                                else:
                                    nc.vector.tensor_tensor(
                                        s_t[:, :], gx[:, t, :], gr[:, t, :],
                                        OP.add)
                                    nc.scalar.activation(
                                        s_t[:, :], s_t[:, :], AF.Prelu,
                                        alpha=SLOPE)

                                # ---- scores + exp ----
                                sc_t = spool.tile([128, H], dt.float32,
                                                  tag="sc")
                                if SIM_SAFE or not STT_SCORES:
                                    tr = spool.tile([128, F], dt.bfloat16,
                                                    tag="trash")
                                    nc.vector.tensor_tensor(
                                        tr[:, :], s_t[:, :], att_t[:, :],
                                        OP.mult)
                                    nc.vector.tensor_reduce(
                                        sc_t[:, :],
                                        tr[:, :].rearrange("p (h c) -> p h c",
                                                           h=H),
                                        mybir.AxisListType.X, OP.add)
                                else:
                                    for h in range(H):
                                        tr = spool.tile([128, 256], dt.bfloat16,
                                                        tag="trash")
                                        nc.vector.scalar_tensor_tensor(
                                            out=tr[:, :],
                                            in0=s_t[:, h * 256:(h + 1) * 256],
                                            scalar=1.0,
                                            in1=att_t[:, h * 256:(h + 1) * 256],
                                            op0=OP.mult, op1=OP.mult,
                                            accum_out=sc_t[:, h:h + 1])
                                if DEBUG and l == 0 and w == 0 and t == 0:
                                    nc.sync.dma_start(out=dbg_s[:, :],
                                                      in_=s_t[:, :])
                                    nc.sync.dma_start(out=dbg_sc[:, :],
                                                      in_=sc_t[:, :])
                                    nc.sync.dma_start(out=dbg_gx[:, :],
                                                      in_=gx[:, 0, :])
                                    nc.sync.dma_start(out=dbg_xr[:, :],
                                                      in_=xr_sb[:, 0, :])
                                    nc.sync.dma_start(
                                        out=dbg_mT[:, :],
                                        in_=maskT_t[:, 0:128])
                                nc.scalar.activation(
                                    ex_w[:, t, :], sc_t[:, :], AF.Exp)
                                nc.scalar.copy(exb_w[:, t, :], ex_w[:, t, :])

                                # ---- alphaM = emask * ex (per head) ----
                                aM = ampool.tile([128, H, 128], dt.bfloat16,
                                                 tag="aM")
                                aM_w.append(aM)
                                for h in range(H):
                                    nc.vector.tensor_scalar(
                                        aM[:, h, :],
                                        mask_t[:, t * 128:(t + 1) * 128],
                                        ex_w[:, t, h:h + 1], None, OP.mult)

                                # ---- denominator + aggregation ----
                                nc.tensor.matmul(
                                    ps_den[:, :],
                                    mask_t[:, t * 128:(t + 1) * 128],
                                    exb_w[:, t, :], start=(t == 0),
                                    stop=(t == T - 1))
                                if concat:
                                    for h in range(H):
                                        nc.tensor.matmul(
                                            ps_g[h][:, :],
                                            aM[:, h, :],
                                            gx[:, t, h * 256:(h + 1) * 256],
                                            start=(t == 0), stop=(t == T - 1))

                            # ---- window epilogue ----
                            den_t = spool.tile([128, H], dt.float32, tag="wden")
                            nc.vector.tensor_scalar(den_t[:, :], ps_den[:, :H],
                                                    float(EPS), None, OP.add)
                            rec_t = spool.tile([128, H], dt.float32, tag="wrec")
                            nc.vector.reciprocal(rec_t[:, :], den_t[:, :])
                            if concat:
                                hn = spool.tile([128, F], dt.bfloat16, tag="hn")
                                for h in range(H):
                                    nc.scalar.activation(
                                        hn[:, h * 256:(h + 1) * 256],
                                        ps_g[h][:, :], AF.Copy,
                                        scale=rec_t[:, h:h + 1])
                                nc.vector.tensor_tensor(hn[:, :], hn[:, :],
                                                        bias_t[:, :], OP.add)
                                # elu: max(x, exp(min(x,0)) - 1)
                                mm = spool.tile([128, F], dt.bfloat16,
                                                tag="elu_m")
                                nc.vector.tensor_scalar(mm[:, :], hn[:, :], 0.0,
                                                        None, OP.min)
                                nc.scalar.activation(mm[:, :], mm[:, :], AF.Exp)
                                hb = spool.tile([128, F], dt.bfloat16, tag="hb")
                                nc.vector.scalar_tensor_tensor(
                                    hb[:, :], mm[:, :], -1.0, hn[:, :],
                                    OP.add, OP.max)
                                nc.sync.dma_start(
                                    out=h_dram[l][w * 128:(w + 1) * 128, :],
                                    in_=hb[:, :])
                            else:
                                # mean over heads (fold 1/H into rec);
                                # aggregate in two 3-head passes to fit PSUM
                                rec6 = spool.tile([128, H], dt.float32,
                                                  tag="rec6")
                                nc.vector.tensor_scalar(rec6[:, :], rec_t[:, :],
                                                        1.0 / H, None, OP.mult)
                                acc = spool.tile([128, 256], dt.float32,
                                                 tag="acc")
                                for gi, grp in enumerate(((0, 1, 2),
                                                          (3, 4, 5))):
                                    for t in range(T):
                                        for j, h in enumerate(grp):
                                            nc.tensor.matmul(
                                                ps_g[j][:, :],
                                                aM_w[t][:, h, :],
                                                gx[:, t,
                                                   h * 256:(h + 1) * 256],
                                                start=(t == 0),
                                                stop=(t == T - 1))
                                    for j, h in enumerate(grp):
                                        if h == 0:
                                            nc.vector.tensor_scalar(
                                                acc[:, :], ps_g[j][:, :],
                                                rec6[:, 0:1], None, OP.mult)
                                        else:
                                            nc.vector.scalar_tensor_tensor(
                                                acc[:, :], ps_g[j][:, :],
                                                rec6[:, h:h + 1], acc[:, :],
                                                OP.mult, OP.add)
                                nc.vector.tensor_tensor(acc[:, :], acc[:, :],
                                                        bias_t[:, :], OP.add)
                                # l2 normalize rows
                                ss = spool.tile([128, 1], dt.float32, tag="ss")
                                trash2 = spool.tile([128, 256], dt.float32,
                                                    tag="trash2")
                                if SIM_SAFE:
                                    nc.vector.tensor_tensor(
                                        trash2[:, :], acc[:, :], acc[:, :],
                                        OP.mult)
                                    nc.vector.tensor_reduce(
                                        ss[:, :], trash2[:, :],
                                        mybir.AxisListType.X, OP.add)
                                else:
                                    nc.vector.scalar_tensor_tensor(
                                        trash2[:, :], acc[:, :], 1.0, acc[:, :],
                                        OP.mult, OP.mult, accum_out=ss[:, :])
                                nrm = spool.tile([128, 1], dt.float32,
                                                 tag="nrm")
                                nc.scalar.activation(nrm[:, :], ss[:, :],
                                                     AF.Sqrt)
                                nc.vector.tensor_scalar(nrm[:, :], nrm[:, :],
                                                        1e-12, None, OP.max)
                                rn = spool.tile([128, 1], dt.float32, tag="rn")
                                nc.vector.reciprocal(rn[:, :], nrm[:, :])
                                hb = spool.tile([128, 256], dt.bfloat16,
                                                tag="hb")
                                nc.vector.tensor_scalar(hb[:, :], acc[:, :],
                                                        rn[:, :], None, OP.mult)
                                # pool: pooled_T[c, g] += sum_n h[n,c] pmask[n,g]
                                for b in range(2):
                                    nc.tensor.matmul(
                                        pool_ps[b][:, :],
                                        hb[:, b * 128:(b + 1) * 128],
                                        pmask_t[:, w * N_GRAPHS:
                                                (w + 1) * N_GRAPHS],
                                        start=(w == 0), stop=(w == NWIN - 1))

                        if l == 2:
                            for b in range(2):
                                pl = auxpool.tile([128, N_GRAPHS], dt.float32,
                                                  tag="pl")
                                nc.vector.tensor_copy(pl[:, :],
                                                      pool_ps[b][:, :])
                                nc.sync.dma_start(
                                    out=pool_loc[b * 128:(b + 1) * 128, :],
                                    in_=pl[:, :])

            # ---- D: pooled -> AllReduce -> MLP ----
            with (
                tc.tile_pool(name="mlp", bufs=1) as mpool,
                tc.tile_pool(name="psM", bufs=1, space="PSUM") as psM,
            ):
                nc.gpsimd.collective_compute(
                    "AllReduce", mybir.AluOpType.add, replica_groups=rg,
                    ins=[pool_loc.ap().opt()],
                    outs=[pool_full.ap().opt()])

                rcnt_t = mpool.tile([128, N_GRAPHS], dt.float32)
                nc.sync.dma_start(out=rcnt_t[:, :], in_=rcnt_in[:, :])
                pz = mpool.tile([128, 2, N_GRAPHS], dt.bfloat16)
                for b in range(2):
                    pf = mpool.tile([128, N_GRAPHS], dt.float32, tag="pf")
                    nc.sync.dma_start(out=pf[:, :],
                                      in_=pool_full[b * 128:(b + 1) * 128, :])
                    nc.vector.tensor_tensor(pz[:, b, :], pf[:, :],
                                            rcnt_t[:, :], OP.mult)

                wfc1_t = mpool.tile([128, 2, 256], dt.bfloat16)
                wfc2_t = mpool.tile([128, 2, 768], dt.bfloat16)
                for b in range(2):
                    nc.sync.dma_start(out=wfc1_t[:, b, :],
                                      in_=wfc1_in[:, b * 256:(b + 1) * 256])
                    nc.sync.dma_start(out=wfc2_t[:, b, :],
                                      in_=wfc2_in[:, b * 768:(b + 1) * 768])
                bfc1_t = mpool.tile([128, 2], dt.float32)
                nc.sync.dma_start(out=bfc1_t[:, :], in_=bfc1_in[:, :])
                bfc2_t = mpool.tile([128, 768], dt.float32)
                nc.sync.dma_start(out=bfc2_t[:, :], in_=bfc2_in[:, :])

                z1 = mpool.tile([128, 2, N_GRAPHS], dt.bfloat16)
                for it in range(2):
                    ps1 = psM.tile([128, N_GRAPHS], dt.float32, tag="ps1")
                    for b in range(2):
                        nc.tensor.matmul(
                            ps1[:, :],
                            wfc1_t[:, b, it * 128:(it + 1) * 128],
                            pz[:, b, :], start=(b == 0), stop=(b == 1))
                    nc.scalar.activation(z1[:, it, :], ps1[:, :], AF.Relu,
                                         bias=bfc1_t[:, it:it + 1], scale=1.0)

                for gt in range(N_GRAPHS // 128):
                    ps2 = psM.tile([128, 768], dt.float32, tag="ps2")
                    for jc, (j0, jw) in enumerate(((0, 512), (512, 256))):
                        for b in range(2):
                            nc.tensor.matmul(
                                ps2[:, j0:j0 + jw],
                                z1[:, b, gt * 128:(gt + 1) * 128],
                                wfc2_t[:, b, j0:j0 + jw],
                                start=(b == 0), stop=(b == 1))
                    zo = mpool.tile([128, 768], dt.float32, tag="zo")
                    nc.vector.tensor_tensor(zo[:, :], ps2[:, :],
                                            bfc2_t[:, :], OP.add)
                    nc.sync.dma_start(
                        out=out_ext[gt * 128:(gt + 1) * 128, :], in_=zo[:, :])

    nc.compile()
    return nc


def kernel(**inputs):
    T, in_maps = _preprocess(inputs)
    if T not in _PROG_CACHE:
        _PROG_CACHE[T] = _build(T)
    nc = _PROG_CACHE[T]
    r = run_bass_kernel_spmd(nc, in_maps, list(range(NCORES)), trace=False)
    return r.results[0]["out"]


# revision 31
# speedup vs baseline: 1.2839x; 1.1506x over previous
"""GATv2 (3-layer, PyG semantics) + global mean pool + MLP on 8 trn2 NeuronCores.

Self-contained: hardcodes problem shapes from nn_GAT_47906065219807.
Sharding: data-parallel over contiguous node ranges (2500 nodes/core); each
core owns edges whose dst lands in its range (edges cross shards via an
AllGather of the source-side projections).

v2 edge phase:
  - layers 0/1: gr is expanded from the xr window tile by a PE matmul with a
    transposed edge mask, and gx is added in-PSUM via an identity matmul;
    ACT applies Prelu straight from PSUM (no DVE add, no gr gather).
  - softmax weights are folded into the aggregation matmul's stationary mask
    (alphaM = emask * ex) built by one broadcast DVE op per tile; the
    division by the softmax denominator happens per-node after aggregation.
  - att-dot via fused tensor_tensor_reduce per head.
"""
import numpy as np
import ml_dtypes

import concourse.bacc as bacc
import concourse.mybir as mybir
import concourse.tile as tile
from concourse.bass_utils import run_bass_kernel_spmd

# problem constants
N_NODES = 20000
N_EDGES = 120000
N_GRAPHS = 512
F_IN = 300
NHID = 256
NOUT = 768
SLOPE = 0.2
EPS = 1e-16

NCORES = 8
NLOC = N_NODES // NCORES            # 2500
NPAD = 2560                         # 20 tiles of 128
NWIN = NPAD // 128                  # 20 windows / node tiles per core
KIN_PAD = 384                       # F_IN padded to 3*128

# per-layer dims: (K_in_padded, F_out, heads, concat)
LAYERS = [
    (KIN_PAD, 4 * NHID, 4, True),
    (4 * NHID, 4 * NHID, 4, True),
    (4 * NHID, 6 * NHID, 6, False),
]

_BF = ml_dtypes.bfloat16
_PROG_CACHE = {}
SIM_SAFE = False      # sim lacks the fused DVE-accumulator writeback
STT_SCORES = True     # fused per-head stt+accum scores (else mult+3D-reduce)
BCAST_ALPHAM = True   # single broadcast-TT alphaM per tile (else per-head)
DEBUG = False


def _bf16(a):
    return np.ascontiguousarray(a.astype(_BF)).view(np.uint16)


def _wrap_idx(flat_idx):
    """dma_gather index layout: slot i -> [i % 16, i // 16], replicated to
    128 partitions (8 Q7 cores x 16)."""
    n = flat_idx.shape[0]
    assert n % 16 == 0
    w = flat_idx.reshape(n // 16, 16).T.astype(np.int16)
    return np.tile(w, (8, 1)).copy()


def _preprocess(inputs):
    """Host-side sharding/layout. Returns (T, in_maps)."""
    x = np.asarray(inputs["x"], np.float32)
    ei = np.asarray(inputs["edge_index"]).astype(np.int64)
    batch = np.asarray(inputs["batch"]).astype(np.int64)

    loops = np.arange(N_NODES, dtype=np.int64)
    src = np.concatenate([ei[0], loops])
    dst = np.concatenate([ei[1], loops])

    # padded-global row index into the AllGather output [NCORES*NPAD, F]
    src_pad = (src // NLOC) * NPAD + (src % NLOC)

    core_of = dst // NLOC
    # per (core, window) edge buckets
    buckets = [[[] for _ in range(NWIN)] for _ in range(NCORES)]
    order = np.argsort(dst, kind="stable")
    for e in order:
        c = core_of[e]
        dl = dst[e] - c * NLOC
        buckets[c][dl // 128].append(e)
    Tw = tuple(
        max((len(buckets[c][w]) + 127) // 128 for c in range(NCORES))
        for w in range(NWIN)
    )
    Soff = np.concatenate([[0], np.cumsum([t * 128 for t in Tw])])

    # graph counts -> reciprocal (reference divides by max(cnt,1))
    cnt = np.bincount(batch, minlength=N_GRAPHS).astype(np.float32)
    rcnt = 1.0 / np.maximum(cnt, 1.0)

    # shared (per-core identical) tensors
    def wT_pad(w, kpad):
        # host layout [128, KB, F]: [p, b, f] = w[f-major...]  w is [F, K]
        wt = w.T.astype(np.float32)                    # [K, F]
        K, F = wt.shape
        out = np.zeros((kpad, F), np.float32)
        out[:K] = wt
        return _bf16(out.reshape(kpad // 128, 128, F).transpose(1, 0, 2)
                     .reshape(128, (kpad // 128) * F))

    shared = {
        "w1l": wT_pad(np.asarray(inputs["c1_wl"]), KIN_PAD),
        "w1r": wT_pad(np.asarray(inputs["c1_wr"]), KIN_PAD),
        "w2l": wT_pad(np.asarray(inputs["c2_wl"]), 4 * NHID),
        "w2r": wT_pad(np.asarray(inputs["c2_wr"]), 4 * NHID),
        "w3l": wT_pad(np.asarray(inputs["c3_wl"]), 4 * NHID),
        "w3r": wT_pad(np.asarray(inputs["c3_wr"]), 4 * NHID),
        "att1": _bf16(np.tile(np.asarray(inputs["c1_att"]).reshape(1, -1), (128, 1))),
        "att2": _bf16(np.tile(np.asarray(inputs["c2_att"]).reshape(1, -1), (128, 1))),
        "att3": _bf16(np.tile(np.asarray(inputs["c3_att"]).reshape(1, -1), (128, 1))),
        "b1": np.tile(np.asarray(inputs["c1_b"], np.float32).reshape(1, -1), (128, 1)),
        "b2": np.tile(np.asarray(inputs["c2_b"], np.float32).reshape(1, -1), (128, 1)),
        "b3": np.tile(np.asarray(inputs["c3_b"], np.float32).reshape(1, -1), (128, 1)),
        "rcnt": np.tile(rcnt.reshape(1, -1), (128, 1)).astype(np.float32),
        "wfc1": wT_pad(np.asarray(inputs["fp1_w"]), 256),
        "wfc2": wT_pad(np.asarray(inputs["fp2_w"]), 256),
        "bfc1": np.asarray(inputs["fp1_b"], np.float32).reshape(2, 128).T.copy(),
        "bfc2": np.tile(np.asarray(inputs["fp2_b"], np.float32).reshape(1, -1),
                        (128, 1)),
        "ident": _bf16(np.eye(128, dtype=np.float32)),
    }

    in_maps = []
    for c in range(NCORES):
        xc = np.zeros((NPAD, KIN_PAD), np.float32)
        xc[:NLOC, :F_IN] = x[c * NLOC:(c + 1) * NLOC]
        # xT host layout [128, 3, NPAD]
        xT = xc.T.reshape(KIN_PAD // 128, 128, NPAD).transpose(1, 0, 2)
        xT = _bf16(xT.reshape(128, (KIN_PAD // 128) * NPAD))

        tot = int(Soff[-1])
        isrc = np.zeros(tot, np.int64)
        idst = np.zeros(tot, np.int64)
        emask = np.zeros((128, tot), np.float32)
        emaskT = np.zeros((128, tot), np.float32)
        for w in range(NWIN):
            es = buckets[c][w]
            s0 = int(Soff[w])
            for i, e in enumerate(es):
                isrc[s0 + i] = src_pad[e]
                idst[s0 + i] = dst[e] - c * NLOC
                n = (dst[e] - c * NLOC) - w * 128
                emask[i % 128, s0 + (i // 128) * 128 + n] = 1.0
                emaskT[n, s0 + (i // 128) * 128 + (i % 128)] = 1.0
        pmask = np.zeros((128, NWIN * N_GRAPHS), np.float32)
        bl = batch[c * NLOC:(c + 1) * NLOC]
        for nl in range(NLOC):
            pmask[nl % 128, (nl // 128) * N_GRAPHS + bl[nl]] = 1.0

        m = dict(shared)
        m["xT"] = xT
        m["isrc"] = _wrap_idx(isrc)
        m["idst"] = _wrap_idx(idst)
        m["emask"] = _bf16(emask)
        m["emaskT"] = _bf16(emaskT)
        m["pmask"] = _bf16(pmask)
        in_maps.append(m)
    return Tw, in_maps


def _build(Tw):
    Tw = tuple(Tw)
    TMAX = max(Tw)
    Soff = [0]
    for t in Tw:
        Soff.append(Soff[-1] + t * 128)
    TOT = Soff[-1]
    nc = bacc.Bacc("TRN2", target_bir_lowering=False, debug=False,
                   num_devices=NCORES)
    dt = mybir.dt
    AF = mybir.ActivationFunctionType
    OP = mybir.AluOpType

    def inp(name, shape, d):
        return nc.dram_tensor(name, shape, d, kind="ExternalInput")

    xT_in = inp("xT", [128, (KIN_PAD // 128) * NPAD], dt.bfloat16)
    isrc_in = inp("isrc", [128, TOT // 16], dt.int16)
    idst_in = inp("idst", [128, TOT // 16], dt.int16)
    emask_in = inp("emask", [128, TOT], dt.bfloat16)
    emaskT_in = inp("emaskT", [128, TOT], dt.bfloat16)
    pmask_in = inp("pmask", [128, NWIN * N_GRAPHS], dt.bfloat16)
    ident_in = inp("ident", [128, 128], dt.bfloat16)
    w_in = [(inp("w1l", [128, 3 * 1024], dt.bfloat16),
             inp("w1r", [128, 3 * 1024], dt.bfloat16)),
            (inp("w2l", [128, 8 * 1024], dt.bfloat16),
             inp("w2r", [128, 8 * 1024], dt.bfloat16)),
            (inp("w3l", [128, 8 * 1536], dt.bfloat16),
             inp("w3r", [128, 8 * 1536], dt.bfloat16))]
    att_in = [inp("att1", [128, 1024], dt.bfloat16),
              inp("att2", [128, 1024], dt.bfloat16),
              inp("att3", [128, 1536], dt.bfloat16)]
    b_in = [inp("b1", [128, 1024], dt.float32),
            inp("b2", [128, 1024], dt.float32),
            inp("b3", [128, 256], dt.float32)]
    rcnt_in = inp("rcnt", [128, N_GRAPHS], dt.float32)
    wfc1_in = inp("wfc1", [128, 2 * 256], dt.bfloat16)
    wfc2_in = inp("wfc2", [128, 2 * 768], dt.bfloat16)
    bfc1_in = inp("bfc1", [128, 2], dt.float32)
    bfc2_in = inp("bfc2", [128, 768], dt.float32)
    out_ext = nc.dram_tensor("out", [N_GRAPHS, NOUT], dt.float32,
                             kind="ExternalOutput")
    if DEBUG:
        dbg_z = nc.dram_tensor("dbg_z", [128, 1024], dt.float32,
                               kind="ExternalOutput")
        dbg_s = nc.dram_tensor("dbg_s", [128, 1024], dt.bfloat16,
                               kind="ExternalOutput")
        dbg_sc = nc.dram_tensor("dbg_sc", [128, 4], dt.float32,
                                kind="ExternalOutput")
        dbg_gx = nc.dram_tensor("dbg_gx", [128, 1024], dt.bfloat16,
                                kind="ExternalOutput")
        dbg_xr = nc.dram_tensor("dbg_xr", [128, 1024], dt.bfloat16,
                                kind="ExternalOutput")
        dbg_mT = nc.dram_tensor("dbg_mT", [128, 128], dt.bfloat16,
                                kind="ExternalOutput")

    # internal DRAM
    xl_loc = [nc.dram_tensor(f"xl_loc{l}", [NPAD, F], dt.bfloat16)
              for l, (_, F, _, _) in enumerate(LAYERS)]
    xr_loc2 = nc.dram_tensor("xr_loc2", [NPAD, 1536], dt.bfloat16)
    xl_full = [nc.dram_tensor(f"xl_full{l}", [NCORES * NPAD, F], dt.bfloat16,
                              addr_space="Shared")
               for l, (_, F, _, _) in enumerate(LAYERS)]
    h_dram = [nc.dram_tensor(f"h_dram{l}", [NPAD, 1024], dt.bfloat16)
              for l in range(2)]
    pool_loc = nc.dram_tensor("pool_loc", [256, N_GRAPHS], dt.float32)
    pool_full = nc.dram_tensor("pool_full", [256, N_GRAPHS], dt.float32,
                               addr_space="Shared")

    rg = [list(range(NCORES))]

    with tile.TileContext(nc) as tc:
        with (
            tc.tile_pool(name="persist", bufs=1) as ppool,
        ):
            isrc_t = ppool.tile([128, TOT // 16], dt.int16)
            nc.sync.dma_start(out=isrc_t[:, :], in_=isrc_in[:, :])
            idst_t = ppool.tile([128, TOT // 16], dt.int16)
            nc.sync.dma_start(out=idst_t[:, :], in_=idst_in[:, :])
            ident_t = ppool.tile([128, 128], dt.bfloat16)
            nc.sync.dma_start(out=ident_t[:, :], in_=ident_in[:, :])

            pool_ps = [None, None]

            for l, (K, F, H, concat) in enumerate(LAYERS):
                KB = K // 128
                NCH = F // 512          # 512-col chunks of F
                expand = True           # gr via PE mask-expand + in-PSUM add

                with tc.tile_pool(name=f"xr{l}", bufs=1) as xrpool:
                    if expand:
                        xr_sb = xrpool.tile([128, NWIN, F], dt.bfloat16)

                    with (
                        tc.tile_pool(name=f"w{l}", bufs=1) as wpool,
                        tc.tile_pool(name=f"hT{l}", bufs=1) as hpool,
                        tc.tile_pool(name=f"mm{l}", bufs=4) as mmpool,
                        tc.tile_pool(name=f"psA{l}", bufs=2,
                                     space="PSUM") as psA,
                    ):
                        # ---- load hT (layer input, [128, KB, NPAD] bf16) ----
                        hT = hpool.tile([128, KB, NPAD], dt.bfloat16, tag="hT")
                        if l == 0:
                            for b in range(KB):
                                nc.sync.dma_start(
                                    out=hT[:, b, :],
                                    in_=xT_in[:, b * NPAD:(b + 1) * NPAD])
                        else:
                            # quarter blocks so this overlaps the previous
                            # layer's edge phase (h windows become ready
                            # incrementally)
                            QN = NPAD // 4
                            for q in range(4):
                                for b in range(KB):
                                    nc.sync.dma_start(
                                        out=hT[:, b, q * QN:(q + 1) * QN],
                                        in_=h_dram[l - 1][q * QN:(q + 1) * QN,
                                                          b * 128:(b + 1) * 128],
                                        transpose=True)

                        # ---- weights ----
                        wl_t = wpool.tile([128, KB, F], dt.bfloat16)
                        wr_t = wpool.tile([128, KB, F], dt.bfloat16)
                        for wt, win in ((wl_t, w_in[l][0]), (wr_t, w_in[l][1])):
                            for b in range(KB):
                                nc.sync.dma_start(
                                    out=wt[:, b, :],
                                    in_=win[:, b * F:(b + 1) * F])

                        # ---- A: projections xl = h @ wl.T, xr = h @ wr.T ----
                        for side, wt in ((0, wl_t), (1, wr_t)):
                            if side == 1:
                                # AllGather xl overlaps with the xr projection
                                nc.gpsimd.collective_compute(
                                    "AllGather", mybir.AluOpType.bypass,
                                    replica_groups=rg,
                                    ins=[xl_loc[l].ap().opt()],
                                    outs=[xl_full[l].ap().opt()])
                            for t in range(NWIN):
                                for ch in range(NCH):
                                    ps = psA.tile([128, 512], dt.float32,
                                                  tag="mmps")
                                    for b in range(KB):
                                        nc.tensor.matmul(
                                            ps[:, :],
                                            hT[:, b, t * 128:(t + 1) * 128],
                                            wt[:, b, ch * 512:(ch + 1) * 512],
                                            start=(b == 0), stop=(b == KB - 1))
                                    if side == 1 and expand:
                                        nc.scalar.copy(
                                            xr_sb[:, t, ch * 512:(ch + 1) * 512],
                                            ps[:, :])
                                    else:
                                        ob = mmpool.tile([128, 512], dt.bfloat16,
                                                         tag="mmout")
                                        nc.scalar.copy(ob[:, :], ps[:, :])
                                        dst_dram = (xl_loc[l] if side == 0
                                                    else xr_loc2)
                                        nc.sync.dma_start(
                                            out=dst_dram[t * 128:(t + 1) * 128,
                                                         ch * 512:(ch + 1) * 512],
                                            in_=ob[:, :])

                    # ---- C: edge phase ----
                    with (
                        tc.tile_pool(name=f"g{l}", bufs=3 if l < 2 else 2) as gpool,
                        tc.tile_pool(name=f"ew{l}", bufs=2) as epool,
                        tc.tile_pool(name=f"es{l}", bufs=3) as spool,
                        tc.tile_pool(name=f"am{l}",
                                     bufs=(3 if l < 2 else TMAX + 1)) as ampool,
                        tc.tile_pool(name=f"psZ{l}", bufs=2,
                                     space="PSUM") as psZ,
                        tc.tile_pool(name=f"psE{l}", bufs=1,
                                     space="PSUM") as psE,
                        tc.tile_pool(name=f"psD{l}", bufs=1,
                                     space="PSUM") as psD,
                        tc.tile_pool(name=f"psP{l}", bufs=1,
                                     space="PSUM") as psPool,
                        tc.tile_pool(name=f"aux{l}", bufs=1) as auxpool,
                    ):
                        att_t = auxpool.tile([128, F], dt.bfloat16)
                        nc.sync.dma_start(out=att_t[:, :], in_=att_in[l][:, :])
                        bias_t = auxpool.tile([128, F if concat else 256],
                                              dt.float32)
                        nc.sync.dma_start(out=bias_t[:, :], in_=b_in[l][:, :])
                        if l == 2:
                            pmask_t = auxpool.tile([128, NWIN * N_GRAPHS],
                                                   dt.bfloat16)
                            nc.sync.dma_start(out=pmask_t[:, :],
                                              in_=pmask_in[:, :])
                            pool_ps[0] = psPool.tile([128, N_GRAPHS], dt.float32,
                                                     tag="poolps0",
                                                     name="poolps0")
                            pool_ps[1] = psPool.tile([128, N_GRAPHS], dt.float32,
                                                     tag="poolps1",
                                                     name="poolps1")

                        for w in range(NWIN):
                            T = Tw[w]
                            S = T * 128
                            mask_t = epool.tile([128, TMAX * 128], dt.bfloat16,
                                                tag="emask")
                            nc.sync.dma_start(
                                out=mask_t[:, :S],
                                in_=emask_in[:, Soff[w]:Soff[w + 1]])
                            gx = gpool.tile([128, TMAX, F], dt.bfloat16,
                                            tag="gx")
                            nc.gpsimd.dma_gather(
                                gx[:, :T, :], xl_full[l][:, :],
                                isrc_t[:, Soff[w] // 16:Soff[w + 1] // 16],
                                num_idxs=S, num_idxs_reg=S, elem_size=F)
                            if expand:
                                maskT_t = epool.tile([128, TMAX * 128],
                                                     dt.bfloat16, tag="emaskT")
                                nc.sync.dma_start(
                                    out=maskT_t[:, :S],
                                    in_=emaskT_in[:, Soff[w]:Soff[w + 1]])
                            else:
                                gr = gpool.tile([128, TMAX, F], dt.bfloat16,
                                                tag="gr")
                                nc.gpsimd.dma_gather(
                                    gr[:, :T, :], xr_loc2[:, :],
                                    idst_t[:, Soff[w] // 16:Soff[w + 1] // 16],
                                    num_idxs=S, num_idxs_reg=S, elem_size=F)

                            ex_w = spool.tile([128, TMAX, H], dt.float32,
                                              tag="exw")
                            exb_w = spool.tile([128, TMAX, H], dt.bfloat16,
                                               tag="exbw")
                            ps_den = psD.tile([128, H], dt.float32, tag="den")
                            NAGG = H if concat else H // 2
                            ps_g = [psE.tile([128, 256], dt.float32,
                                             tag=f"agg{j}", name=f"agg{j}")
                                    for j in range(NAGG)]

                            aM_w = []
                            for t in range(T):
                                # ---- s = prelu(xl[src] + xr[dst]) ----
                                s_t = spool.tile([128, F], dt.bfloat16, tag="s")
                                if expand:
                                    ps_z = None
                                    for ch in range(NCH):
                                        ps_zc = psZ.tile([128, 512],
                                                         dt.float32, tag="z")
                                        nc.tensor.matmul(
                                            ps_zc[:, :],
                                            maskT_t[:, t * 128:(t + 1) * 128],
                                            xr_sb[:, w, ch * 512:(ch + 1) * 512],
                                            start=True, stop=False)
                                        nc.tensor.matmul(
                                            ps_zc[:, :],
                                            ident_t[:, :],
                                            gx[:, t, ch * 512:(ch + 1) * 512],
                                            start=False, stop=True)
                                        nc.scalar.activation(
                                            s_t[:, ch * 512:(ch + 1) * 512],
                                            ps_zc[:, :], AF.Prelu,
                                            alpha=SLOPE)
                                else:
                                    nc.vector.tensor_tensor(
                                        s_t[:, :], gx[:, t, :], gr[:, t, :],
                                        OP.add)
                                    nc.scalar.activation(
                                        s_t[:, :], s_t[:, :], AF.Prelu,
                                        alpha=SLOPE)

                                # ---- scores + exp ----
                                sc_t = spool.tile([128, H], dt.float32,
                                                  tag="sc")
                                if SIM_SAFE or not STT_SCORES:
                                    tr = spool.tile([128, F], dt.bfloat16,
                                                    tag="trash")
                                    nc.vector.tensor_tensor(
                                        tr[:, :], s_t[:, :], att_t[:, :],
                                        OP.mult)
                                    nc.vector.tensor_reduce(
                                        sc_t[:, :],
                                        tr[:, :].rearrange("p (h c) -> p h c",
                                                           h=H),
                                        mybir.AxisListType.X, OP.add)
                                else:
                                    for h in range(H):
                                        tr = spool.tile([128, 256], dt.bfloat16,
                                                        tag="trash")
                                        nc.vector.scalar_tensor_tensor(
                                            out=tr[:, :],
                                            in0=s_t[:, h * 256:(h + 1) * 256],
                                            scalar=1.0,
                                            in1=att_t[:, h * 256:(h + 1) * 256],
                                            op0=OP.mult, op1=OP.mult,
                                            accum_out=sc_t[:, h:h + 1])
                                if DEBUG and l == 0 and w == 0 and t == 0:
                                    nc.sync.dma_start(out=dbg_s[:, :],
                                                      in_=s_t[:, :])
                                    nc.sync.dma_start(out=dbg_sc[:, :],
                                                      in_=sc_t[:, :])
                                    nc.sync.dma_start(out=dbg_gx[:, :],
                                                      in_=gx[:, 0, :])
                                    nc.sync.dma_start(out=dbg_xr[:, :],
                                                      in_=xr_sb[:, 0, :])
                                    nc.sync.dma_start(
                                        out=dbg_mT[:, :],
                                        in_=maskT_t[:, 0:128])
                                nc.scalar.activation(
                                    ex_w[:, t, :], sc_t[:, :], AF.Exp)
                                nc.scalar.copy(exb_w[:, t, :], ex_w[:, t, :])

                                # ---- alphaM = emask * ex ----
                                aM = ampool.tile([128, H, 128], dt.bfloat16,
                                                 tag="aM")
                                aM_w.append(aM)
                                if BCAST_ALPHAM:
                                    em_b = (mask_t[:, t * 128:(t + 1) * 128]
                                            .unsqueeze(1)
                                            .broadcast_to([128, H, 128]))
                                    ex_b = (exb_w[:, t, :].unsqueeze(2)
                                            .broadcast_to([128, H, 128]))
                                    nc.vector.tensor_tensor(
                                        aM[:, :, :], em_b, ex_b, OP.mult)
                                else:
                                    for h in range(H):
                                        nc.vector.tensor_scalar(
                                            aM[:, h, :],
                                            mask_t[:, t * 128:(t + 1) * 128],
                                            ex_w[:, t, h:h + 1], None, OP.mult)

                                # ---- denominator + aggregation ----
                                nc.tensor.matmul(
                                    ps_den[:, :],
                                    mask_t[:, t * 128:(t + 1) * 128],
                                    exb_w[:, t, :], start=(t == 0),
                                    stop=(t == T - 1))
                                if concat:
                                    for h in range(H):
                                        nc.tensor.matmul(
                                            ps_g[h][:, :],
                                            aM[:, h, :],
                                            gx[:, t, h * 256:(h + 1) * 256],
                                            start=(t == 0), stop=(t == T - 1))

                            # ---- window epilogue ----
                            den_t = spool.tile([128, H], dt.float32, tag="wden")
                            nc.vector.tensor_scalar(den_t[:, :], ps_den[:, :H],
                                                    float(EPS), None, OP.add)
                            rec_t = spool.tile([128, H], dt.float32, tag="wrec")
                            nc.vector.reciprocal(rec_t[:, :], den_t[:, :])
                            if concat:
                                hn = spool.tile([128, F], dt.bfloat16, tag="hn")
                                for h in range(H):
                                    nc.scalar.activation(
                                        hn[:, h * 256:(h + 1) * 256],
                                        ps_g[h][:, :], AF.Copy,
                                        scale=rec_t[:, h:h + 1])
                                nc.vector.tensor_tensor(hn[:, :], hn[:, :],
                                                        bias_t[:, :], OP.add)
                                # elu: max(x, exp(min(x,0)) - 1)
                                mm = spool.tile([128, F], dt.bfloat16,
                                                tag="elu_m")
                                nc.vector.tensor_scalar(mm[:, :], hn[:, :], 0.0,
                                                        None, OP.min)
                                nc.scalar.activation(mm[:, :], mm[:, :], AF.Exp)
                                hb = spool.tile([128, F], dt.bfloat16, tag="hb")
                                nc.vector.scalar_tensor_tensor(
                                    hb[:, :], mm[:, :], -1.0, hn[:, :],
                                    OP.add, OP.max)
                                nc.sync.dma_start(
                                    out=h_dram[l][w * 128:(w + 1) * 128, :],
                                    in_=hb[:, :])
                            else:
                                # mean over heads (fold 1/H into rec);
                                # aggregate in two 3-head passes to fit PSUM
                                rec6 = spool.tile([128, H], dt.float32,
                                                  tag="rec6")
                                nc.vector.tensor_scalar(rec6[:, :], rec_t[:, :],
                                                        1.0 / H, None, OP.mult)
                                acc = spool.tile([128, 256], dt.float32,
                                                 tag="acc")
                                for gi, grp in enumerate(((0, 1, 2),
                                                          (3, 4, 5))):
                                    for t in range(T):
                                        for j, h in enumerate(grp):
                                            nc.tensor.matmul(
                                                ps_g[j][:, :],
                                                aM_w[t][:, h, :],
                                                gx[:, t,
                                                   h * 256:(h + 1) * 256],
                                                start=(t == 0),
                                                stop=(t == T - 1))
                                    for j, h in enumerate(grp):
                                        if h == 0:
                                            nc.vector.tensor_scalar(
                                                acc[:, :], ps_g[j][:, :],
                                                rec6[:, 0:1], None, OP.mult)
                                        else:
                                            nc.vector.scalar_tensor_tensor(
                                                acc[:, :], ps_g[j][:, :],
                                                rec6[:, h:h + 1], acc[:, :],
                                                OP.mult, OP.add)
                                nc.vector.tensor_tensor(acc[:, :], acc[:, :],
                                                        bias_t[:, :], OP.add)
                                # l2 normalize rows
                                ss = spool.tile([128, 1], dt.float32, tag="ss")
                                trash2 = spool.tile([128, 256], dt.float32,
                                                    tag="trash2")
                                if SIM_SAFE:
                                    nc.vector.tensor_tensor(
                                        trash2[:, :], acc[:, :], acc[:, :],
                                        OP.mult)
                                    nc.vector.tensor_reduce(
                                        ss[:, :], trash2[:, :],
                                        mybir.AxisListType.X, OP.add)
                                else:
                                    nc.vector.scalar_tensor_tensor(
                                        trash2[:, :], acc[:, :], 1.0, acc[:, :],
                                        OP.mult, OP.mult, accum_out=ss[:, :])
                                nrm = spool.tile([128, 1], dt.float32,
                                                 tag="nrm")
                                nc.scalar.activation(nrm[:, :], ss[:, :],
                                                     AF.Sqrt)
                                nc.vector.tensor_scalar(nrm[:, :], nrm[:, :],
                                                        1e-12, None, OP.max)
                                rn = spool.tile([128, 1], dt.float32, tag="rn")
                                nc.vector.reciprocal(rn[:, :], nrm[:, :])
                                hb = spool.tile([128, 256], dt.bfloat16,
                                                tag="hb")
                                nc.vector.tensor_scalar(hb[:, :], acc[:, :],
                                                        rn[:, :], None, OP.mult)
                                # pool: pooled_T[c, g] += sum_n h[n,c] pmask[n,g]
                                for b in range(2):
                                    nc.tensor.matmul(
                                        pool_ps[b][:, :],
                                        hb[:, b * 128:(b + 1) * 128],
                                        pmask_t[:, w * N_GRAPHS:
                                                (w + 1) * N_GRAPHS],
                                        start=(w == 0), stop=(w == NWIN - 1))

                        if l == 2:
                            for b in range(2):
                                pl = auxpool.tile([128, N_GRAPHS], dt.float32,
                                                  tag="pl")
                                nc.vector.tensor_copy(pl[:, :],
                                                      pool_ps[b][:, :])
                                nc.sync.dma_start(
                                    out=pool_loc[b * 128:(b + 1) * 128, :],
                                    in_=pl[:, :])

            # ---- D: pooled -> AllReduce -> MLP ----
            with (
                tc.tile_pool(name="mlp", bufs=1) as mpool,
                tc.tile_pool(name="psM", bufs=1, space="PSUM") as psM,
            ):
                nc.gpsimd.collective_compute(
                    "AllReduce", mybir.AluOpType.add, replica_groups=rg,
                    ins=[pool_loc.ap().opt()],
                    outs=[pool_full.ap().opt()])

                rcnt_t = mpool.tile([128, N_GRAPHS], dt.float32)
                nc.sync.dma_start(out=rcnt_t[:, :], in_=rcnt_in[:, :])
                pz = mpool.tile([128, 2, N_GRAPHS], dt.bfloat16)
                for b in range(2):
                    pf = mpool.tile([128, N_GRAPHS], dt.float32, tag="pf")
                    nc.sync.dma_start(out=pf[:, :],
                                      in_=pool_full[b * 128:(b + 1) * 128, :])
                    nc.vector.tensor_tensor(pz[:, b, :], pf[:, :],
                                            rcnt_t[:, :], OP.mult)

                wfc1_t = mpool.tile([128, 2, 256], dt.bfloat16)
                wfc2_t = mpool.tile([128, 2, 768], dt.bfloat16)
                for b in range(2):
                    nc.sync.dma_start(out=wfc1_t[:, b, :],
                                      in_=wfc1_in[:, b * 256:(b + 1) * 256])
                    nc.sync.dma_start(out=wfc2_t[:, b, :],
                                      in_=wfc2_in[:, b * 768:(b + 1) * 768])
                bfc1_t = mpool.tile([128, 2], dt.float32)
                nc.sync.dma_start(out=bfc1_t[:, :], in_=bfc1_in[:, :])
                bfc2_t = mpool.tile([128, 768], dt.float32)
                nc.sync.dma_start(out=bfc2_t[:, :], in_=bfc2_in[:, :])

                z1 = mpool.tile([128, 2, N_GRAPHS], dt.bfloat16)
                for it in range(2):
                    ps1 = psM.tile([128, N_GRAPHS], dt.float32, tag="ps1")
                    for b in range(2):
                        nc.tensor.matmul(
                            ps1[:, :],
                            wfc1_t[:, b, it * 128:(it + 1) * 128],
                            pz[:, b, :], start=(b == 0), stop=(b == 1))
                    nc.scalar.activation(z1[:, it, :], ps1[:, :], AF.Relu,
                                         bias=bfc1_t[:, it:it + 1], scale=1.0)

                for gt in range(N_GRAPHS // 128):
                    ps2 = psM.tile([128, 768], dt.float32, tag="ps2")
                    for jc, (j0, jw) in enumerate(((0, 512), (512, 256))):
                        for b in range(2):
                            nc.tensor.matmul(
                                ps2[:, j0:j0 + jw],
                                z1[:, b, gt * 128:(gt + 1) * 128],
                                wfc2_t[:, b, j0:j0 + jw],
                                start=(b == 0), stop=(b == 1))
                    zo = mpool.tile([128, 768], dt.float32, tag="zo")
                    nc.vector.tensor_tensor(zo[:, :], ps2[:, :],
                                            bfc2_t[:, :], OP.add)
                    nc.sync.dma_start(
                        out=out_ext[gt * 128:(gt + 1) * 128, :], in_=zo[:, :])

    nc.compile()
    return nc


def kernel(**inputs):
    T, in_maps = _preprocess(inputs)
    if T not in _PROG_CACHE:
        _PROG_CACHE[T] = _build(T)
    nc = _PROG_CACHE[T]
    r = run_bass_kernel_spmd(nc, in_maps, list(range(NCORES)), trace=False)
    return r.results[0]["out"]
